# revision 22
# baseline (speedup 1.0000x reference)
"""Switch-style top-1 MoE layer on 8 Trainium2 NeuronCores.

Strategy (expert parallelism, as in the torch module's dist.all_to_all):
  - host: router (X @ Wr, argmax, softmax top prob) — 50 MFLOP, trivial
  - host: dispatch — sort tokens by assigned expert, pad each expert's
    token set to a common capacity C, pre-transpose to [D, C]
  - device: core i holds expert i's weights and computes
    Y_i.T = W2.T @ relu(W1.T @ X_i.T + b1) + b2 entirely in
    [feature, token] layout
  - host: combine — un-transpose, scatter back to token order, scale by
    the router prob.

Device kernel layout notes:
  - W1 is pre-swizzled on the host to m-major [MF, 128, KD*128] so each
    128-wide d_ff tile arrives in ONE contiguous DMA and the first
    matmul only waits for ~0.2 MB, not all of W1.
  - Streaming blocks interleave layer-1 and layer-2 m-tiles (software-
    pipelined by one stage) with KD pinned layer-2 PSUM groups, so the
    W1/W2 DMA demand is spread across the whole block and weight tiles
    are consumed in arrival order; the last block runs layer 2 d-outer
    so the final evictions + output DMAs overlap the remaining matmuls.
  - DMA issues are spread across Sync/Scalar/GpSimd sequencers so the
    per-issue descriptor cost doesn't serialize ahead of the first tile
    (and the Scalar engine stays free for PSUM evictions).
  - Matmul dtype: float32r (full-rate fp32, ~2e-4 rel err) or bfloat16
    (~3e-3 rel err, fast weight load) via MOE_DTYPE=fp32r|bf16.

All shapes hardcoded for B=2, S=2048, D=768, E=8, F=3072.
"""

import os
import sys

if "/opt/trn_rl_repo" not in sys.path:
    sys.path.insert(0, "/opt/trn_rl_repo")

import numpy as np

B, S, D = 2, 2048, 768
E, F = 8, 3072
P = 128
KD = D // P   # 6  k-tiles over d_model
MF = F // P   # 24 tiles over d_ff

# filled by the most recent kernel() call when MOE_TRACE=1 (test.py reads it)
LAST_PROFILE = {}


def _install_trace_shims():
    """Enable NTFF profiling under axon: inject the antenv.axon_hooks module
    that trn_boot expects, and disable artifact upload (zero-egress box)."""
    import types

    if "antenv.axon_hooks" not in sys.modules:
        hooks = types.ModuleType("antenv.axon_hooks")
        hooks._hook = None
        hooks.set_axon_ntff_profile_hook = lambda h: setattr(hooks, "_hook", h)
        hooks.get_axon_ntff_profile_hook = lambda: hooks._hook
        import antenv

        antenv.axon_hooks = hooks
        sys.modules["antenv.axon_hooks"] = hooks
        from trn_agent_boot.trn_boot import _ntff_profile_via_ctypes

        hooks.set_axon_ntff_profile_hook(
            _ntff_profile_via_ctypes("/opt/axon/libaxon_pjrt.so")
        )
    import concourse.bass_utils as bass_utils

    bass_utils.upload_artifacts = lambda tmpdir: "(local)"


def _blocks_for(C):
    """Split C token columns into matmul moving-dim blocks.

    Each block must be <=512 (one fp32 PSUM bank); blocks >=256 keep fp32r
    matmuls at full rate (1 cycle/row)."""
    nb = -(-C // 512)
    base = (C // nb) & ~7
    sizes = [base] * (nb - 1) + [C - base * (nb - 1)]
    blocks = []
    off = 0
    for tb in sizes:
        blocks.append((off, tb))
        off += tb
    return blocks


_NC_CACHE = {}


def _fast_drain_and_barrier(self, tick_clock, wait_clock):
    """Tile kernel epilogue without the ~250-semaphore one-by-one clear
    storm (~7 us on the longest engine chain). The NEFF's initial runtime
    handshake re-initializes semaphore state on every execution, so the
    end-of-kernel clears only matter for back-to-back executions without a
    runtime reset — verified safe by repeated kernel() calls in-process."""
    from concourse.vector_clock import ScopedClock

    drain_inst = self.nc.sync.drain()
    wait_clock.add_sem_waits(
        drain_inst.ins, ScopedClock({None: tick_clock.global_clock})
    )
    self.nc.all_engine_barrier()
    popped = self.nc._tile_sem_poison_stack.pop()
    assert popped is self._sem_poison


def _build_device_kernel(C, blocks, wdt_name):
    import concourse.mybir as mybir
    import concourse.tile as tile
    from concourse import bacc

    key = (C, tuple(blocks), wdt_name)
    if key in _NC_CACHE:
        return _NC_CACHE[key]

    WDT = mybir.dt.float32r if wdt_name == "fp32r" else mybir.dt.bfloat16
    F32 = mybir.dt.float32
    TBmax = max(tb for _, tb in blocks)
    last_bi = len(blocks) - 1

    nc = bacc.Bacc("TRN2", target_bir_lowering=False, debug=False,
                   enable_asserts=False)
    xt_d = nc.dram_tensor("xt", [D, C], WDT, kind="ExternalInput").ap()
    # m-major swizzle: w1m[m, p, k*128 + c] = W1[k*128 + p, m*128 + c]
    w1_d = nc.dram_tensor("w1m", [MF, P, KD * P], WDT, kind="ExternalInput").ap()
    b1_d = nc.dram_tensor("b1", [P, MF], F32, kind="ExternalInput").ap()
    w2_d = nc.dram_tensor("w2", [F, D], WDT, kind="ExternalInput").ap()
    b2_d = nc.dram_tensor("b2", [P, KD], F32, kind="ExternalInput").ap()
    yt_d = nc.dram_tensor("yt", [D, C], F32, kind="ExternalOutput").ap()

    tile.TileContext._drain_and_barrier = _fast_drain_and_barrier
    with tile.TileContext(nc) as tc:
        with tc.tile_pool(name="cpool", bufs=1) as cpool, \
             tc.tile_pool(name="ypool", bufs=3) as ypool, \
             tc.tile_pool(name="ps1", bufs=2, space="PSUM") as ps1, \
             tc.tile_pool(name="ps2", bufs=6, space="PSUM") as ps2:
            b1_t = cpool.tile([P, MF], F32, tag="b1")
            b2_t = cpool.tile([P, KD], F32, tag="b2")

            # per-m-tile weight/activation tiles: fine-grained DMA deps so
            # compute starts as soon as the first tiles land. The first two
            # m-tiles are further split per-k (32KB slices) so the very first
            # matmuls start ~3us earlier, overlapping the DMA stream ramp.
            N_SPLIT = 2
            w1_kt = {(m, k): cpool.tile([P, P], WDT, tag=f"w1k_{m}_{k}",
                                        name=f"w1k_{m}_{k}")
                     for m in range(N_SPLIT) for k in range(KD)}
            w1_t = [None if m < N_SPLIT else
                    cpool.tile([P, KD * P], WDT, tag=f"w1_{m}", name=f"w1_{m}")
                    for m in range(MF)]

            def w1_slice(m, k):
                if m < N_SPLIT:
                    return w1_kt[(m, k)][:]
                return w1_t[m][:, k * P:(k + 1) * P]
            w2_t = [cpool.tile([P, D], WDT, tag=f"w2_{m}", name=f"w2_{m}")
                    for m in range(MF)]
            xt_t = {}
            for bi, (off, TB) in enumerate(blocks):
                for k in range(KD):
                    xt_t[(bi, k)] = cpool.tile([P, TB], WDT, tag=f"xt_{bi}_{k}",
                                               name=f"xt_{bi}_{k}")

            # DMA issue order = consumption order, spread over three
            # sequencers so issue cost doesn't serialize the critical path:
            #   sync:   W1 m-tiles (layer-1 weight stream)
            #   scalar: biases only (scalar must stay free for evictions)
            #   gpsimd: activations
            # W1/W2 m-tiles interleaved to match the streaming block's
            # pipelined consumption order (w1_m, then w2_{m-1})
            for k in range(KD):
                nc.sync.dma_start(out=w1_kt[(0, k)][:],
                                  in_=w1_d[0][:, k * P:(k + 1) * P])
            for m in range(1, MF):
                if m < N_SPLIT:
                    for k in range(KD):
                        nc.sync.dma_start(out=w1_kt[(m, k)][:],
                                          in_=w1_d[m][:, k * P:(k + 1) * P])
                else:
                    nc.sync.dma_start(out=w1_t[m][:], in_=w1_d[m])
                nc.sync.dma_start(out=w2_t[m - 1][:],
                                  in_=w2_d[(m - 1) * P:m * P, :])
            nc.sync.dma_start(out=w2_t[MF - 1][:],
                              in_=w2_d[(MF - 1) * P:MF * P, :])
            nc.scalar.dma_start(out=b1_t[:], in_=b1_d[:])
            nc.scalar.dma_start(out=b2_t[:], in_=b2_d[:])
            for bi, (off, TB) in enumerate(blocks):
                for k in range(KD):
                    nc.gpsimd.dma_start(out=xt_t[(bi, k)][:],
                                        in_=xt_d[k * P:(k + 1) * P, off:off + TB])
            def l1_group(bi, off, TB, ht_t, m):
                ps = ps1.tile([P, TBmax], F32, tag="ps1", name=f"ps1_{bi}_{m}")
                for k in range(KD):
                    nc.tensor.matmul(
                        ps[:, :TB],
                        w1_slice(m, k),
                        xt_t[(bi, k)][:],
                        start=(k == 0), stop=(k == KD - 1),
                    )
                nc.scalar.activation(
                    ht_t[m][:], ps[:, :TB],
                    mybir.ActivationFunctionType.Relu,
                    bias=b1_t[:, m:m + 1],
                )

            def l2_group(TB, ht_t, ps_o, m):
                for d in range(KD):
                    nc.tensor.matmul(
                        ps_o[d][:, :TB],
                        w2_t[m][:, d * P:(d + 1) * P],
                        ht_t[m][:],
                        start=(m == 0), stop=(m == MF - 1),
                    )

            for bi, (off, TB) in enumerate(blocks):
                ht_t = [cpool.tile([P, TB], WDT, tag=f"ht_{m}", name=f"ht_{bi}_{m}")
                        for m in range(MF)]
                if bi < last_bi:
                    # Streaming block: interleave layer-1 and layer-2 m-tiles
                    # (software-pipelined by one stage so the ReLU eviction
                    # hides) — spreads the W1+W2 DMA demand over the whole
                    # block so the PE isn't paced by the weight stream.
                    # Layer-2 accumulates into KD pinned PSUM groups.
                    ps_o = [ps2.tile([P, TBmax], F32, tag="ps2",
                                     name=f"ps2_{bi}_{d}") for d in range(KD)]
                    for m in range(MF):
                        l1_group(bi, off, TB, ht_t, m)
                        if m >= 1:
                            l2_group(TB, ht_t, ps_o, m - 1)
                    l2_group(TB, ht_t, ps_o, MF - 1)
                    for d in range(KD):
                        y_t = ypool.tile([P, TBmax], F32, tag="yt")
                        nc.vector.tensor_scalar_add(y_t[:, :TB], ps_o[d][:, :TB],
                                                    b2_t[:, d:d + 1])
                        nc.sync.dma_start(out=yt_d[d * P:(d + 1) * P, off:off + TB],
                                          in_=y_t[:, :TB])
                else:
                    # Final block (weights already resident): layer 1, then
                    # d-outer layer 2 so each d-group finishes early and the
                    # evictions + output DMAs overlap the remaining matmuls.
                    for m in range(MF):
                        l1_group(bi, off, TB, ht_t, m)
                    for d in range(KD):
                        ps_o = ps2.tile([P, TBmax], F32, tag="ps2",
                                        name=f"ps2L_{d}")
                        for m in range(MF):
                            nc.tensor.matmul(
                                ps_o[:, :TB],
                                w2_t[m][:, d * P:(d + 1) * P],
                                ht_t[m][:],
                                start=(m == 0), stop=(m == MF - 1),
                            )
                        y_t = ypool.tile([P, TBmax], F32, tag="yt")
                        nc.vector.tensor_scalar_add(y_t[:, :TB], ps_o[:, :TB],
                                                    b2_t[:, d:d + 1])
                        nc.sync.dma_start(out=yt_d[d * P:(d + 1) * P, off:off + TB],
                                          in_=y_t[:, :TB])

    nc.compile()
    _NC_CACHE[key] = nc
    return nc


def kernel(hidden_states, Wr, W1, b1, W2, b2):
    trace = os.environ.get("MOE_TRACE") == "1"
    wdt_name = os.environ.get("MOE_DTYPE", "bf16")
    if trace:
        _install_trace_shims()

    import ml_dtypes
    from concourse.bass_utils import run_bass_kernel_spmd

    wnp = np.float32 if wdt_name == "fp32r" else ml_dtypes.bfloat16

    X = np.ascontiguousarray(np.asarray(hidden_states, np.float32).reshape(B * S, D))
    Wr = np.asarray(Wr, np.float32)
    W1 = np.asarray(W1, np.float32)
    b1 = np.asarray(b1, np.float32)
    W2 = np.asarray(W2, np.float32)
    b2 = np.asarray(b2, np.float32)
    T = B * S

    # --- router (replicated; host) ---
    logits = X @ Wr                                   # [T, E] fp32
    expert_index = logits.argmax(-1)
    # top-1 softmax prob, computed the same way jax.nn.softmax does (max-shift)
    top = logits.max(-1, keepdims=True)
    prob = 1.0 / np.exp(logits - top, dtype=np.float32).sum(-1)  # [T]

    # --- dispatch: group tokens by expert ---
    order = np.argsort(expert_index, kind="stable")
    sorted_eidx = expert_index[order]
    bounds = np.searchsorted(sorted_eidx, np.arange(E + 1))
    counts = np.diff(bounds)
    C = max(256, int(-(-counts.max() // 8)) * 8)
    blocks = _blocks_for(C)

    Xs = X[order]
    in_maps = []
    for i in range(E):
        lo, hi = bounds[i], bounds[i + 1]
        xt = np.zeros((D, C), wnp)
        xt[:, : hi - lo] = Xs[lo:hi].T
        # m-major swizzle of W1: [D, F] -> [MF, 128, KD*128]
        w1m = np.ascontiguousarray(
            W1[i].reshape(KD, P, MF, P).transpose(2, 1, 0, 3).reshape(MF, P, KD * P)
        ).astype(wnp)
        in_maps.append({
            "xt": xt,
            "w1m": w1m,
            "b1": np.ascontiguousarray(b1[i].reshape(MF, P).T),
            "w2": np.ascontiguousarray(W2[i]).astype(wnp),
            "b2": np.ascontiguousarray(b2[i].reshape(KD, P).T),
        })

    # --- expert FFNs on the 8 cores ---
    nc = _build_device_kernel(C, blocks, wdt_name)
    res = run_bass_kernel_spmd(nc, in_maps, core_ids=list(range(E)), trace=trace)
    if trace:
        LAST_PROFILE.clear()
        LAST_PROFILE.update(
            exec_time_ns=res.exec_time_ns,
            mean_exec_time_ns=res.mean_exec_time_ns,
            max_exec_time_core_id=res.max_exec_time_core_id,
            trace_path=(res.instructions_and_trace or (None, None))[1],
            profile_json=res.profile_json,
            capacity=C,
            blocks=blocks,
            counts=counts.tolist(),
            dtype=wdt_name,
        )

    # --- combine: un-sort, scale by router prob ---
    Ys = np.empty((T, D), np.float32)
    for i in range(E):
        lo, hi = bounds[i], bounds[i + 1]
        Ys[lo:hi] = res.results[i]["yt"][:, : hi - lo].T
    combined = np.empty_like(Ys)
    combined[order] = Ys
    out = (prob[:, None] * combined).reshape(B, S, D)

    return (
        out,
        logits.reshape(B, S, E),
        expert_index.reshape(B, S).astype(np.int32),
    )



# revision 23
# speedup vs baseline: 1.0170x; 1.0170x over previous
"""Switch-style top-1 MoE layer on 8 Trainium2 NeuronCores.

Strategy (expert parallelism, as in the torch module's dist.all_to_all):
  - host: router (X @ Wr, argmax, softmax top prob) — 50 MFLOP, trivial
  - host: dispatch — sort tokens by assigned expert, pad each expert's
    token set to a common capacity C, pre-transpose to [D, C]
  - device: core i holds expert i's weights and computes
    Y_i.T = W2.T @ relu(W1.T @ X_i.T + b1) + b2 entirely in
    [feature, token] layout
  - host: combine — un-transpose, scatter back to token order, scale by
    the router prob.

Device kernel layout notes:
  - W1 is pre-swizzled on the host to m-major [MF, 128, KD*128] so each
    128-wide d_ff tile arrives in ONE contiguous DMA and the first
    matmul only waits for ~0.2 MB, not all of W1.
  - Streaming blocks interleave layer-1 and layer-2 m-tiles (software-
    pipelined by one stage) with KD pinned layer-2 PSUM groups, so the
    W1/W2 DMA demand is spread across the whole block and weight tiles
    are consumed in arrival order; the last block runs layer 2 d-outer
    so the final evictions + output DMAs overlap the remaining matmuls.
  - DMA issues are spread across Sync/Scalar/GpSimd sequencers so the
    per-issue descriptor cost doesn't serialize ahead of the first tile
    (and the Scalar engine stays free for PSUM evictions).
  - Matmul dtype: float32r (full-rate fp32, ~2e-4 rel err) or bfloat16
    (~3e-3 rel err, fast weight load) via MOE_DTYPE=fp32r|bf16.

All shapes hardcoded for B=2, S=2048, D=768, E=8, F=3072.
"""

import os
import sys

if "/opt/trn_rl_repo" not in sys.path:
    sys.path.insert(0, "/opt/trn_rl_repo")

import numpy as np

B, S, D = 2, 2048, 768
E, F = 8, 3072
P = 128
KD = D // P   # 6  k-tiles over d_model
MF = F // P   # 24 tiles over d_ff

# filled by the most recent kernel() call when MOE_TRACE=1 (test.py reads it)
LAST_PROFILE = {}


def _install_trace_shims():
    """Enable NTFF profiling under axon: inject the antenv.axon_hooks module
    that trn_boot expects, and disable artifact upload (zero-egress box)."""
    import types

    if "antenv.axon_hooks" not in sys.modules:
        hooks = types.ModuleType("antenv.axon_hooks")
        hooks._hook = None
        hooks.set_axon_ntff_profile_hook = lambda h: setattr(hooks, "_hook", h)
        hooks.get_axon_ntff_profile_hook = lambda: hooks._hook
        import antenv

        antenv.axon_hooks = hooks
        sys.modules["antenv.axon_hooks"] = hooks
        from trn_agent_boot.trn_boot import _ntff_profile_via_ctypes

        hooks.set_axon_ntff_profile_hook(
            _ntff_profile_via_ctypes("/opt/axon/libaxon_pjrt.so")
        )
    import concourse.bass_utils as bass_utils

    bass_utils.upload_artifacts = lambda tmpdir: "(local)"


def _blocks_for(C):
    """Split C token columns into matmul moving-dim blocks.

    Each block must be <=512 (one fp32 PSUM bank); blocks >=256 keep fp32r
    matmuls at full rate (1 cycle/row)."""
    nb = -(-C // 512)
    base = (C // nb) & ~7
    sizes = [base] * (nb - 1) + [C - base * (nb - 1)]
    blocks = []
    off = 0
    for tb in sizes:
        blocks.append((off, tb))
        off += tb
    return blocks


_NC_CACHE = {}


def _fast_drain_and_barrier(self, tick_clock, wait_clock):
    """Tile kernel epilogue without the ~250-semaphore one-by-one clear
    storm (~7 us on the longest engine chain). The NEFF's initial runtime
    handshake re-initializes semaphore state on every execution, so the
    end-of-kernel clears only matter for back-to-back executions without a
    runtime reset — verified safe by repeated kernel() calls in-process."""
    from concourse.vector_clock import ScopedClock

    drain_inst = self.nc.sync.drain()
    wait_clock.add_sem_waits(
        drain_inst.ins, ScopedClock({None: tick_clock.global_clock})
    )
    # No final all-engine barrier: each engine's stream ends after its last
    # real instruction, so the compiler-appended per-engine semaphore resets
    # on early-finishing engines overlap the Tensor engine's trailing work.
    # The barrier's gather/release sems are balanced (remain 0) without it.
    popped = self.nc._tile_sem_poison_stack.pop()
    assert popped is self._sem_poison


def _build_device_kernel(C, blocks, wdt_name):
    import concourse.mybir as mybir
    import concourse.tile as tile
    from concourse import bacc

    key = (C, tuple(blocks), wdt_name)
    if key in _NC_CACHE:
        return _NC_CACHE[key]

    WDT = mybir.dt.float32r if wdt_name == "fp32r" else mybir.dt.bfloat16
    F32 = mybir.dt.float32
    TBmax = max(tb for _, tb in blocks)
    last_bi = len(blocks) - 1

    nc = bacc.Bacc("TRN2", target_bir_lowering=False, debug=False,
                   enable_asserts=False)
    xt_d = nc.dram_tensor("xt", [D, C], WDT, kind="ExternalInput").ap()
    # m-major swizzle: w1m[m, p, k*128 + c] = W1[k*128 + p, m*128 + c]
    w1_d = nc.dram_tensor("w1m", [MF, P, KD * P], WDT, kind="ExternalInput").ap()
    b1_d = nc.dram_tensor("b1", [P, MF], F32, kind="ExternalInput").ap()
    w2_d = nc.dram_tensor("w2", [F, D], WDT, kind="ExternalInput").ap()
    b2_d = nc.dram_tensor("b2", [P, KD], F32, kind="ExternalInput").ap()
    yt_d = nc.dram_tensor("yt", [D, C], F32, kind="ExternalOutput").ap()

    tile.TileContext._drain_and_barrier = _fast_drain_and_barrier
    with tile.TileContext(nc) as tc:
        with tc.tile_pool(name="cpool", bufs=1) as cpool, \
             tc.tile_pool(name="ypool", bufs=3) as ypool, \
             tc.tile_pool(name="ps1", bufs=2, space="PSUM") as ps1, \
             tc.tile_pool(name="ps2", bufs=6, space="PSUM") as ps2:
            b1_t = cpool.tile([P, MF], F32, tag="b1")
            b2_t = cpool.tile([P, KD], F32, tag="b2")

            # per-m-tile weight/activation tiles: fine-grained DMA deps so
            # compute starts as soon as the first tiles land. The first two
            # m-tiles are further split per-k (32KB slices) so the very first
            # matmuls start ~3us earlier, overlapping the DMA stream ramp.
            N_SPLIT = 2
            w1_kt = {(m, k): cpool.tile([P, P], WDT, tag=f"w1k_{m}_{k}",
                                        name=f"w1k_{m}_{k}")
                     for m in range(N_SPLIT) for k in range(KD)}
            w1_t = [None if m < N_SPLIT else
                    cpool.tile([P, KD * P], WDT, tag=f"w1_{m}", name=f"w1_{m}")
                    for m in range(MF)]

            def w1_slice(m, k):
                if m < N_SPLIT:
                    return w1_kt[(m, k)][:]
                return w1_t[m][:, k * P:(k + 1) * P]
            w2_t = [cpool.tile([P, D], WDT, tag=f"w2_{m}", name=f"w2_{m}")
                    for m in range(MF)]
            xt_t = {}
            for bi, (off, TB) in enumerate(blocks):
                for k in range(KD):
                    xt_t[(bi, k)] = cpool.tile([P, TB], WDT, tag=f"xt_{bi}_{k}",
                                               name=f"xt_{bi}_{k}")

            # DMA issue order = consumption order, spread over three
            # sequencers so issue cost doesn't serialize the critical path:
            #   sync:   W1 m-tiles (layer-1 weight stream)
            #   scalar: biases only (scalar must stay free for evictions)
            #   gpsimd: activations
            # W1/W2 m-tiles interleaved to match the streaming block's
            # pipelined consumption order (w1_m, then w2_{m-1})
            for k in range(KD):
                nc.sync.dma_start(out=w1_kt[(0, k)][:],
                                  in_=w1_d[0][:, k * P:(k + 1) * P])
            for m in range(1, MF):
                if m < N_SPLIT:
                    for k in range(KD):
                        nc.sync.dma_start(out=w1_kt[(m, k)][:],
                                          in_=w1_d[m][:, k * P:(k + 1) * P])
                else:
                    nc.sync.dma_start(out=w1_t[m][:], in_=w1_d[m])
                nc.sync.dma_start(out=w2_t[m - 1][:],
                                  in_=w2_d[(m - 1) * P:m * P, :])
            nc.sync.dma_start(out=w2_t[MF - 1][:],
                              in_=w2_d[(MF - 1) * P:MF * P, :])
            nc.scalar.dma_start(out=b1_t[:], in_=b1_d[:])
            nc.scalar.dma_start(out=b2_t[:], in_=b2_d[:])
            for bi, (off, TB) in enumerate(blocks):
                for k in range(KD):
                    nc.gpsimd.dma_start(out=xt_t[(bi, k)][:],
                                        in_=xt_d[k * P:(k + 1) * P, off:off + TB])
            def l1_group(bi, off, TB, ht_t, m):
                ps = ps1.tile([P, TBmax], F32, tag="ps1", name=f"ps1_{bi}_{m}")
                for k in range(KD):
                    nc.tensor.matmul(
                        ps[:, :TB],
                        w1_slice(m, k),
                        xt_t[(bi, k)][:],
                        start=(k == 0), stop=(k == KD - 1),
                    )
                nc.scalar.activation(
                    ht_t[m][:], ps[:, :TB],
                    mybir.ActivationFunctionType.Relu,
                    bias=b1_t[:, m:m + 1],
                )

            def l2_group(TB, ht_t, ps_o, m):
                for d in range(KD):
                    nc.tensor.matmul(
                        ps_o[d][:, :TB],
                        w2_t[m][:, d * P:(d + 1) * P],
                        ht_t[m][:],
                        start=(m == 0), stop=(m == MF - 1),
                    )

            for bi, (off, TB) in enumerate(blocks):
                ht_t = [cpool.tile([P, TB], WDT, tag=f"ht_{m}", name=f"ht_{bi}_{m}")
                        for m in range(MF)]
                if bi < last_bi:
                    # Streaming block: interleave layer-1 and layer-2 m-tiles
                    # (software-pipelined by one stage so the ReLU eviction
                    # hides) — spreads the W1+W2 DMA demand over the whole
                    # block so the PE isn't paced by the weight stream.
                    # Layer-2 accumulates into KD pinned PSUM groups.
                    ps_o = [ps2.tile([P, TBmax], F32, tag="ps2",
                                     name=f"ps2_{bi}_{d}") for d in range(KD)]
                    for m in range(MF):
                        l1_group(bi, off, TB, ht_t, m)
                        if m >= 1:
                            l2_group(TB, ht_t, ps_o, m - 1)
                    l2_group(TB, ht_t, ps_o, MF - 1)
                    for d in range(KD):
                        y_t = ypool.tile([P, TBmax], F32, tag="yt")
                        nc.vector.tensor_scalar_add(y_t[:, :TB], ps_o[d][:, :TB],
                                                    b2_t[:, d:d + 1])
                        nc.sync.dma_start(out=yt_d[d * P:(d + 1) * P, off:off + TB],
                                          in_=y_t[:, :TB])
                else:
                    # Final block (weights already resident): layer 1, then
                    # d-outer layer 2 so each d-group finishes early and the
                    # evictions + output DMAs overlap the remaining matmuls.
                    for m in range(MF):
                        l1_group(bi, off, TB, ht_t, m)
                    for d in range(KD):
                        ps_o = ps2.tile([P, TBmax], F32, tag="ps2",
                                        name=f"ps2L_{d}")
                        for m in range(MF):
                            nc.tensor.matmul(
                                ps_o[:, :TB],
                                w2_t[m][:, d * P:(d + 1) * P],
                                ht_t[m][:],
                                start=(m == 0), stop=(m == MF - 1),
                            )
                        y_t = ypool.tile([P, TBmax], F32, tag="yt")
                        nc.vector.tensor_scalar_add(y_t[:, :TB], ps_o[:, :TB],
                                                    b2_t[:, d:d + 1])
                        nc.sync.dma_start(out=yt_d[d * P:(d + 1) * P, off:off + TB],
                                          in_=y_t[:, :TB])

    nc.compile()
    _NC_CACHE[key] = nc
    return nc


def kernel(hidden_states, Wr, W1, b1, W2, b2):
    trace = os.environ.get("MOE_TRACE") == "1"
    wdt_name = os.environ.get("MOE_DTYPE", "bf16")
    if trace:
        _install_trace_shims()

    import ml_dtypes
    from concourse.bass_utils import run_bass_kernel_spmd

    wnp = np.float32 if wdt_name == "fp32r" else ml_dtypes.bfloat16

    X = np.ascontiguousarray(np.asarray(hidden_states, np.float32).reshape(B * S, D))
    Wr = np.asarray(Wr, np.float32)
    W1 = np.asarray(W1, np.float32)
    b1 = np.asarray(b1, np.float32)
    W2 = np.asarray(W2, np.float32)
    b2 = np.asarray(b2, np.float32)
    T = B * S

    # --- router (replicated; host) ---
    logits = X @ Wr                                   # [T, E] fp32
    expert_index = logits.argmax(-1)
    # top-1 softmax prob, computed the same way jax.nn.softmax does (max-shift)
    top = logits.max(-1, keepdims=True)
    prob = 1.0 / np.exp(logits - top, dtype=np.float32).sum(-1)  # [T]

    # --- dispatch: group tokens by expert ---
    order = np.argsort(expert_index, kind="stable")
    sorted_eidx = expert_index[order]
    bounds = np.searchsorted(sorted_eidx, np.arange(E + 1))
    counts = np.diff(bounds)
    C = max(256, int(counts.max()))
    blocks = _blocks_for(C)

    Xs = X[order]
    in_maps = []
    for i in range(E):
        lo, hi = bounds[i], bounds[i + 1]
        xt = np.zeros((D, C), wnp)
        xt[:, : hi - lo] = Xs[lo:hi].T
        # m-major swizzle of W1: [D, F] -> [MF, 128, KD*128]
        w1m = np.ascontiguousarray(
            W1[i].reshape(KD, P, MF, P).transpose(2, 1, 0, 3).reshape(MF, P, KD * P)
        ).astype(wnp)
        in_maps.append({
            "xt": xt,
            "w1m": w1m,
            "b1": np.ascontiguousarray(b1[i].reshape(MF, P).T),
            "w2": np.ascontiguousarray(W2[i]).astype(wnp),
            "b2": np.ascontiguousarray(b2[i].reshape(KD, P).T),
        })

    # --- expert FFNs on the 8 cores ---
    nc = _build_device_kernel(C, blocks, wdt_name)
    res = run_bass_kernel_spmd(nc, in_maps, core_ids=list(range(E)), trace=trace)
    if trace:
        LAST_PROFILE.clear()
        LAST_PROFILE.update(
            exec_time_ns=res.exec_time_ns,
            mean_exec_time_ns=res.mean_exec_time_ns,
            max_exec_time_core_id=res.max_exec_time_core_id,
            trace_path=(res.instructions_and_trace or (None, None))[1],
            profile_json=res.profile_json,
            capacity=C,
            blocks=blocks,
            counts=counts.tolist(),
            dtype=wdt_name,
        )

    # --- combine: un-sort, scale by router prob ---
    Ys = np.empty((T, D), np.float32)
    for i in range(E):
        lo, hi = bounds[i], bounds[i + 1]
        Ys[lo:hi] = res.results[i]["yt"][:, : hi - lo].T
    combined = np.empty_like(Ys)
    combined[order] = Ys
    out = (prob[:, None] * combined).reshape(B, S, D)

    return (
        out,
        logits.reshape(B, S, E),
        expert_index.reshape(B, S).astype(np.int32),
    )



# revision 24
# speedup vs baseline: 1.0314x; 1.0142x over previous
"""Switch-style top-1 MoE layer on 8 Trainium2 NeuronCores.

Strategy (expert parallelism, as in the torch module's dist.all_to_all):
  - host: router (X @ Wr, argmax, softmax top prob) — 50 MFLOP, trivial
  - host: dispatch — sort tokens by assigned expert, pad each expert's
    token set to a common capacity C, pre-transpose to [D, C]
  - device: core i holds expert i's weights and computes
    Y_i.T = W2.T @ relu(W1.T @ X_i.T + b1) + b2 entirely in
    [feature, token] layout
  - host: combine — un-transpose, scatter back to token order, scale by
    the router prob.

Device kernel layout notes:
  - W1 is pre-swizzled on the host to m-major [MF, 128, KD*128] so each
    128-wide d_ff tile arrives in ONE contiguous DMA and the first
    matmul only waits for ~0.2 MB, not all of W1.
  - Streaming blocks interleave layer-1 and layer-2 m-tiles (software-
    pipelined by one stage) with KD pinned layer-2 PSUM groups, so the
    W1/W2 DMA demand is spread across the whole block and weight tiles
    are consumed in arrival order; the last block runs layer 2 d-outer
    so the final evictions + output DMAs overlap the remaining matmuls.
  - DMA issues are spread across Sync/Scalar/GpSimd sequencers so the
    per-issue descriptor cost doesn't serialize ahead of the first tile
    (and the Scalar engine stays free for PSUM evictions).
  - Matmul dtype: float32r (full-rate fp32, ~2e-4 rel err) or bfloat16
    (~3e-3 rel err, fast weight load) via MOE_DTYPE=fp32r|bf16.

All shapes hardcoded for B=2, S=2048, D=768, E=8, F=3072.
"""

import os
import sys

if "/opt/trn_rl_repo" not in sys.path:
    sys.path.insert(0, "/opt/trn_rl_repo")

import numpy as np

B, S, D = 2, 2048, 768
E, F = 8, 3072
P = 128
KD = D // P   # 6  k-tiles over d_model
MF = F // P   # 24 tiles over d_ff

# filled by the most recent kernel() call when MOE_TRACE=1 (test.py reads it)
LAST_PROFILE = {}


def _install_trace_shims():
    """Enable NTFF profiling under axon: inject the antenv.axon_hooks module
    that trn_boot expects, and disable artifact upload (zero-egress box)."""
    import types

    if "antenv.axon_hooks" not in sys.modules:
        hooks = types.ModuleType("antenv.axon_hooks")
        hooks._hook = None
        hooks.set_axon_ntff_profile_hook = lambda h: setattr(hooks, "_hook", h)
        hooks.get_axon_ntff_profile_hook = lambda: hooks._hook
        import antenv

        antenv.axon_hooks = hooks
        sys.modules["antenv.axon_hooks"] = hooks
        from trn_agent_boot.trn_boot import _ntff_profile_via_ctypes

        hooks.set_axon_ntff_profile_hook(
            _ntff_profile_via_ctypes("/opt/axon/libaxon_pjrt.so")
        )
    import concourse.bass_utils as bass_utils

    bass_utils.upload_artifacts = lambda tmpdir: "(local)"


def _blocks_for(C):
    """Split C token columns into matmul moving-dim blocks.

    Each block must be <=512 (one fp32 PSUM bank); blocks >=256 keep fp32r
    matmuls at full rate (1 cycle/row)."""
    nb = -(-C // 512)
    base = (C // nb) & ~7
    sizes = [base] * (nb - 1) + [C - base * (nb - 1)]
    blocks = []
    off = 0
    for tb in sizes:
        blocks.append((off, tb))
        off += tb
    return blocks


_NC_CACHE = {}


def _fast_drain_and_barrier(self, tick_clock, wait_clock):
    """Tile kernel epilogue without the ~250-semaphore one-by-one clear
    storm (~7 us on the longest engine chain). The NEFF's initial runtime
    handshake re-initializes semaphore state on every execution, so the
    end-of-kernel clears only matter for back-to-back executions without a
    runtime reset — verified safe by repeated kernel() calls in-process."""
    from concourse.vector_clock import ScopedClock

    drain_inst = self.nc.sync.drain()
    wait_clock.add_sem_waits(
        drain_inst.ins, ScopedClock({None: tick_clock.global_clock})
    )
    # No final all-engine barrier: each engine's stream ends after its last
    # real instruction, so the compiler-appended per-engine semaphore resets
    # on early-finishing engines overlap the Tensor engine's trailing work.
    # The barrier's gather/release sems are balanced (remain 0) without it.
    popped = self.nc._tile_sem_poison_stack.pop()
    assert popped is self._sem_poison


def _build_device_kernel(C, blocks, wdt_name):
    import concourse.mybir as mybir
    import concourse.tile as tile
    from concourse import bacc

    key = (C, tuple(blocks), wdt_name)
    if key in _NC_CACHE:
        return _NC_CACHE[key]

    WDT = mybir.dt.float32r if wdt_name == "fp32r" else mybir.dt.bfloat16
    F32 = mybir.dt.float32
    TBmax = max(tb for _, tb in blocks)
    last_bi = len(blocks) - 1

    nc = bacc.Bacc("TRN2", target_bir_lowering=False, debug=False,
                   enable_asserts=False)
    xt_d = nc.dram_tensor("xt", [D, C], WDT, kind="ExternalInput").ap()
    # m-major swizzle: w1m[m, p, k*128 + c] = W1[k*128 + p, m*128 + c]
    w1_d = nc.dram_tensor("w1m", [MF, P, KD * P], WDT, kind="ExternalInput").ap()
    b1_d = nc.dram_tensor("b1", [P, MF], F32, kind="ExternalInput").ap()
    w2_d = nc.dram_tensor("w2", [F, D], WDT, kind="ExternalInput").ap()
    b2_d = nc.dram_tensor("b2", [P, KD], F32, kind="ExternalInput").ap()
    yt_d = nc.dram_tensor("yt", [D, C], F32, kind="ExternalOutput").ap()

    tile.TileContext._drain_and_barrier = _fast_drain_and_barrier
    with tile.TileContext(nc) as tc:
        with tc.tile_pool(name="cpool", bufs=1) as cpool, \
             tc.tile_pool(name="ypool", bufs=3) as ypool, \
             tc.tile_pool(name="ps1", bufs=2, space="PSUM") as ps1, \
             tc.tile_pool(name="ps2", bufs=6, space="PSUM") as ps2:
            b1_t = cpool.tile([P, MF], F32, tag="b1")
            b2_t = cpool.tile([P, KD], F32, tag="b2")

            # per-m-tile weight/activation tiles: fine-grained DMA deps so
            # compute starts as soon as the first tiles land.
            w1_t = [cpool.tile([P, KD * P], WDT, tag=f"w1_{m}", name=f"w1_{m}")
                    for m in range(MF)]

            def w1_slice(m, k):
                return w1_t[m][:, k * P:(k + 1) * P]
            w2_t = [cpool.tile([P, D], WDT, tag=f"w2_{m}", name=f"w2_{m}")
                    for m in range(MF)]
            xt_t = {}
            for bi, (off, TB) in enumerate(blocks):
                for k in range(KD):
                    xt_t[(bi, k)] = cpool.tile([P, TB], WDT, tag=f"xt_{bi}_{k}",
                                               name=f"xt_{bi}_{k}")

            # DMA issue order = consumption order, spread over three
            # sequencers so issue cost doesn't serialize the critical path:
            #   sync:   W1 m-tiles (layer-1 weight stream)
            #   scalar: biases only (scalar must stay free for evictions)
            #   gpsimd: activations
            # W1/W2 m-tiles interleaved to match the streaming block's
            # pipelined consumption order (w1_m, then w2_{m-1})
            nc.sync.dma_start(out=w1_t[0][:], in_=w1_d[0])
            for m in range(1, MF):
                nc.sync.dma_start(out=w1_t[m][:], in_=w1_d[m])
                nc.sync.dma_start(out=w2_t[m - 1][:],
                                  in_=w2_d[(m - 1) * P:m * P, :])
            nc.sync.dma_start(out=w2_t[MF - 1][:],
                              in_=w2_d[(MF - 1) * P:MF * P, :])
            nc.scalar.dma_start(out=b1_t[:], in_=b1_d[:])
            nc.scalar.dma_start(out=b2_t[:], in_=b2_d[:])
            for bi, (off, TB) in enumerate(blocks):
                for k in range(KD):
                    nc.gpsimd.dma_start(out=xt_t[(bi, k)][:],
                                        in_=xt_d[k * P:(k + 1) * P, off:off + TB])
            def l1_group(bi, off, TB, ht_t, m):
                ps = ps1.tile([P, TBmax], F32, tag="ps1", name=f"ps1_{bi}_{m}")
                for k in range(KD):
                    nc.tensor.matmul(
                        ps[:, :TB],
                        w1_slice(m, k),
                        xt_t[(bi, k)][:],
                        start=(k == 0), stop=(k == KD - 1),
                    )
                nc.scalar.activation(
                    ht_t[m][:], ps[:, :TB],
                    mybir.ActivationFunctionType.Relu,
                    bias=b1_t[:, m:m + 1],
                )

            def l2_group(TB, ht_t, ps_o, m):
                for d in range(KD):
                    nc.tensor.matmul(
                        ps_o[d][:, :TB],
                        w2_t[m][:, d * P:(d + 1) * P],
                        ht_t[m][:],
                        start=(m == 0), stop=(m == MF - 1),
                    )

            for bi, (off, TB) in enumerate(blocks):
                ht_t = [cpool.tile([P, TB], WDT, tag=f"ht_{m}", name=f"ht_{bi}_{m}")
                        for m in range(MF)]
                if bi < last_bi:
                    # Streaming block: interleave layer-1 and layer-2 m-tiles
                    # (software-pipelined by one stage so the ReLU eviction
                    # hides) — spreads the W1+W2 DMA demand over the whole
                    # block so the PE isn't paced by the weight stream.
                    # Layer-2 accumulates into KD pinned PSUM groups.
                    ps_o = [ps2.tile([P, TBmax], F32, tag="ps2",
                                     name=f"ps2_{bi}_{d}") for d in range(KD)]
                    for m in range(MF):
                        l1_group(bi, off, TB, ht_t, m)
                        if m >= 1:
                            l2_group(TB, ht_t, ps_o, m - 1)
                    l2_group(TB, ht_t, ps_o, MF - 1)
                    for d in range(KD):
                        y_t = ypool.tile([P, TBmax], F32, tag="yt")
                        nc.vector.tensor_scalar_add(y_t[:, :TB], ps_o[d][:, :TB],
                                                    b2_t[:, d:d + 1])
                        nc.sync.dma_start(out=yt_d[d * P:(d + 1) * P, off:off + TB],
                                          in_=y_t[:, :TB])
                else:
                    # Final block (weights already resident): layer 1, then
                    # d-outer layer 2 so each d-group finishes early and the
                    # evictions + output DMAs overlap the remaining matmuls.
                    for m in range(MF):
                        l1_group(bi, off, TB, ht_t, m)
                    for d in range(KD):
                        ps_o = ps2.tile([P, TBmax], F32, tag="ps2",
                                        name=f"ps2L_{d}")
                        for m in range(MF):
                            nc.tensor.matmul(
                                ps_o[:, :TB],
                                w2_t[m][:, d * P:(d + 1) * P],
                                ht_t[m][:],
                                start=(m == 0), stop=(m == MF - 1),
                            )
                        y_t = ypool.tile([P, TBmax], F32, tag="yt")
                        nc.vector.tensor_scalar_add(y_t[:, :TB], ps_o[:, :TB],
                                                    b2_t[:, d:d + 1])
                        nc.sync.dma_start(out=yt_d[d * P:(d + 1) * P, off:off + TB],
                                          in_=y_t[:, :TB])

    nc.compile()
    _NC_CACHE[key] = nc
    return nc


def kernel(hidden_states, Wr, W1, b1, W2, b2):
    trace = os.environ.get("MOE_TRACE") == "1"
    wdt_name = os.environ.get("MOE_DTYPE", "bf16")
    if trace:
        _install_trace_shims()

    import ml_dtypes
    from concourse.bass_utils import run_bass_kernel_spmd

    wnp = np.float32 if wdt_name == "fp32r" else ml_dtypes.bfloat16

    X = np.ascontiguousarray(np.asarray(hidden_states, np.float32).reshape(B * S, D))
    Wr = np.asarray(Wr, np.float32)
    W1 = np.asarray(W1, np.float32)
    b1 = np.asarray(b1, np.float32)
    W2 = np.asarray(W2, np.float32)
    b2 = np.asarray(b2, np.float32)
    T = B * S

    # --- router (replicated; host) ---
    logits = X @ Wr                                   # [T, E] fp32
    expert_index = logits.argmax(-1)
    # top-1 softmax prob, computed the same way jax.nn.softmax does (max-shift)
    top = logits.max(-1, keepdims=True)
    prob = 1.0 / np.exp(logits - top, dtype=np.float32).sum(-1)  # [T]

    # --- dispatch: group tokens by expert ---
    order = np.argsort(expert_index, kind="stable")
    sorted_eidx = expert_index[order]
    bounds = np.searchsorted(sorted_eidx, np.arange(E + 1))
    counts = np.diff(bounds)
    C = max(256, int(counts.max()))
    blocks = _blocks_for(C)

    Xs = X[order]
    in_maps = []
    for i in range(E):
        lo, hi = bounds[i], bounds[i + 1]
        xt = np.zeros((D, C), wnp)
        xt[:, : hi - lo] = Xs[lo:hi].T
        # m-major swizzle of W1: [D, F] -> [MF, 128, KD*128]
        w1m = np.ascontiguousarray(
            W1[i].reshape(KD, P, MF, P).transpose(2, 1, 0, 3).reshape(MF, P, KD * P)
        ).astype(wnp)
        in_maps.append({
            "xt": xt,
            "w1m": w1m,
            "b1": np.ascontiguousarray(b1[i].reshape(MF, P).T),
            "w2": np.ascontiguousarray(W2[i]).astype(wnp),
            "b2": np.ascontiguousarray(b2[i].reshape(KD, P).T),
        })

    # --- expert FFNs on the 8 cores ---
    nc = _build_device_kernel(C, blocks, wdt_name)
    res = run_bass_kernel_spmd(nc, in_maps, core_ids=list(range(E)), trace=trace)
    if trace:
        LAST_PROFILE.clear()
        LAST_PROFILE.update(
            exec_time_ns=res.exec_time_ns,
            mean_exec_time_ns=res.mean_exec_time_ns,
            max_exec_time_core_id=res.max_exec_time_core_id,
            trace_path=(res.instructions_and_trace or (None, None))[1],
            profile_json=res.profile_json,
            capacity=C,
            blocks=blocks,
            counts=counts.tolist(),
            dtype=wdt_name,
        )

    # --- combine: un-sort, scale by router prob ---
    Ys = np.empty((T, D), np.float32)
    for i in range(E):
        lo, hi = bounds[i], bounds[i + 1]
        Ys[lo:hi] = res.results[i]["yt"][:, : hi - lo].T
    combined = np.empty_like(Ys)
    combined[order] = Ys
    out = (prob[:, None] * combined).reshape(B, S, D)

    return (
        out,
        logits.reshape(B, S, E),
        expert_index.reshape(B, S).astype(np.int32),
    )



# revision 25
# speedup vs baseline: 1.0403x; 1.0086x over previous
"""Switch-style top-1 MoE layer on 8 Trainium2 NeuronCores.

Strategy (expert parallelism, as in the torch module's dist.all_to_all):
  - host: router (X @ Wr, argmax, softmax top prob) — 50 MFLOP, trivial
  - host: dispatch — sort tokens by assigned expert, pad each expert's
    token set to a common capacity C, pre-transpose to [D, C]
  - device: core i holds expert i's weights and computes
    Y_i.T = W2.T @ relu(W1.T @ X_i.T + b1) + b2 entirely in
    [feature, token] layout
  - host: combine — un-transpose, scatter back to token order, scale by
    the router prob.

Device kernel layout notes:
  - W1 is pre-swizzled on the host to m-major [MF, 128, KD*128] so each
    128-wide d_ff tile arrives in ONE contiguous DMA and the first
    matmul only waits for ~0.2 MB, not all of W1.
  - Streaming blocks interleave layer-1 and layer-2 m-tiles (software-
    pipelined by one stage) with KD pinned layer-2 PSUM groups, so the
    W1/W2 DMA demand is spread across the whole block and weight tiles
    are consumed in arrival order; the last block runs layer 2 d-outer
    so the final evictions + output DMAs overlap the remaining matmuls.
  - DMA issues are spread across Sync/Scalar/GpSimd sequencers so the
    per-issue descriptor cost doesn't serialize ahead of the first tile
    (and the Scalar engine stays free for PSUM evictions).
  - Matmul dtype: float32r (full-rate fp32, ~2e-4 rel err) or bfloat16
    (~3e-3 rel err, fast weight load) via MOE_DTYPE=fp32r|bf16.

All shapes hardcoded for B=2, S=2048, D=768, E=8, F=3072.
"""

import os
import sys

if "/opt/trn_rl_repo" not in sys.path:
    sys.path.insert(0, "/opt/trn_rl_repo")

import numpy as np

B, S, D = 2, 2048, 768
E, F = 8, 3072
P = 128
KD = D // P   # 6  k-tiles over d_model
MF = F // P   # 24 tiles over d_ff

# filled by the most recent kernel() call when MOE_TRACE=1 (test.py reads it)
LAST_PROFILE = {}


def _install_trace_shims():
    """Enable NTFF profiling under axon: inject the antenv.axon_hooks module
    that trn_boot expects, and disable artifact upload (zero-egress box)."""
    import types

    if "antenv.axon_hooks" not in sys.modules:
        hooks = types.ModuleType("antenv.axon_hooks")
        hooks._hook = None
        hooks.set_axon_ntff_profile_hook = lambda h: setattr(hooks, "_hook", h)
        hooks.get_axon_ntff_profile_hook = lambda: hooks._hook
        import antenv

        antenv.axon_hooks = hooks
        sys.modules["antenv.axon_hooks"] = hooks
        from trn_agent_boot.trn_boot import _ntff_profile_via_ctypes

        hooks.set_axon_ntff_profile_hook(
            _ntff_profile_via_ctypes("/opt/axon/libaxon_pjrt.so")
        )
    import concourse.bass_utils as bass_utils

    bass_utils.upload_artifacts = lambda tmpdir: "(local)"


def _blocks_for(C):
    """Split C token columns into matmul moving-dim blocks.

    Each block must be <=512 (one fp32 PSUM bank); blocks >=256 keep fp32r
    matmuls at full rate (1 cycle/row)."""
    nb = -(-C // 512)
    base = (C // nb) & ~7
    sizes = [base] * (nb - 1) + [C - base * (nb - 1)]
    blocks = []
    off = 0
    for tb in sizes:
        blocks.append((off, tb))
        off += tb
    return blocks


_NC_CACHE = {}


def _fast_drain_and_barrier(self, tick_clock, wait_clock):
    """Tile kernel epilogue without the ~250-semaphore one-by-one clear
    storm (~7 us on the longest engine chain). The NEFF's initial runtime
    handshake re-initializes semaphore state on every execution, so the
    end-of-kernel clears only matter for back-to-back executions without a
    runtime reset — verified safe by repeated kernel() calls in-process."""
    from concourse.vector_clock import ScopedClock

    drain_inst = self.nc.sync.drain()
    wait_clock.add_sem_waits(
        drain_inst.ins, ScopedClock({None: tick_clock.global_clock})
    )
    # No final all-engine barrier: each engine's stream ends after its last
    # real instruction, so the compiler-appended per-engine semaphore resets
    # on early-finishing engines overlap the Tensor engine's trailing work.
    # The barrier's gather/release sems are balanced (remain 0) without it.
    popped = self.nc._tile_sem_poison_stack.pop()
    assert popped is self._sem_poison


def _build_device_kernel(C, blocks, wdt_name):
    import concourse.mybir as mybir
    import concourse.tile as tile
    from concourse import bacc

    key = (C, tuple(blocks), wdt_name)
    if key in _NC_CACHE:
        return _NC_CACHE[key]

    WDT = mybir.dt.float32r if wdt_name == "fp32r" else mybir.dt.bfloat16
    F32 = mybir.dt.float32
    TBmax = max(tb for _, tb in blocks)
    last_bi = len(blocks) - 1

    nc = bacc.Bacc("TRN2", target_bir_lowering=False, debug=False,
                   enable_asserts=False)
    xt_d = nc.dram_tensor("xt", [D, C], WDT, kind="ExternalInput").ap()
    # m-major swizzle: w1m[m, p, k*128 + c] = W1[k*128 + p, m*128 + c]
    w1_d = nc.dram_tensor("w1m", [MF, P, KD * P], WDT, kind="ExternalInput").ap()
    b1_d = nc.dram_tensor("b1", [P, MF], F32, kind="ExternalInput").ap()
    w2_d = nc.dram_tensor("w2", [F, D], WDT, kind="ExternalInput").ap()
    b2_d = nc.dram_tensor("b2", [P, KD], F32, kind="ExternalInput").ap()
    yt_d = nc.dram_tensor("yt", [D, C], F32, kind="ExternalOutput").ap()

    tile.TileContext._drain_and_barrier = _fast_drain_and_barrier
    with tile.TileContext(nc) as tc:
        with tc.tile_pool(name="cpool", bufs=1) as cpool, \
             tc.tile_pool(name="ypool", bufs=3) as ypool, \
             tc.tile_pool(name="ps1", bufs=2, space="PSUM") as ps1, \
             tc.tile_pool(name="ps2", bufs=6, space="PSUM") as ps2:
            b1_t = cpool.tile([P, MF], F32, tag="b1")
            b2_t = cpool.tile([P, KD], F32, tag="b2")

            # per-m-tile weight/activation tiles: fine-grained DMA deps so
            # compute starts as soon as the first tiles land.
            w1_t = [cpool.tile([P, KD * P], WDT, tag=f"w1_{m}", name=f"w1_{m}")
                    for m in range(MF)]

            def w1_slice(m, k):
                return w1_t[m][:, k * P:(k + 1) * P]
            w2_t = [cpool.tile([P, D], WDT, tag=f"w2_{m}", name=f"w2_{m}")
                    for m in range(MF)]
            xt_t = {}
            for bi, (off, TB) in enumerate(blocks):
                for k in range(KD):
                    xt_t[(bi, k)] = cpool.tile([P, TB], WDT, tag=f"xt_{bi}_{k}",
                                               name=f"xt_{bi}_{k}")

            # DMA issue order = consumption order, spread over three
            # sequencers so issue cost doesn't serialize the critical path:
            #   sync:   W1 m-tiles (layer-1 weight stream)
            #   scalar: biases only (scalar must stay free for evictions)
            #   gpsimd: activations
            # W1/W2 m-tiles interleaved to match the streaming block's
            # pipelined consumption order (w1_m, then w2_{m-1})
            nc.sync.dma_start(out=w1_t[0][:], in_=w1_d[0])
            for m in range(1, MF):
                nc.sync.dma_start(out=w1_t[m][:], in_=w1_d[m])
                nc.sync.dma_start(out=w2_t[m - 1][:],
                                  in_=w2_d[(m - 1) * P:m * P, :])
            nc.sync.dma_start(out=w2_t[MF - 1][:],
                              in_=w2_d[(MF - 1) * P:MF * P, :])
            nc.scalar.dma_start(out=b1_t[:], in_=b1_d[:])
            nc.scalar.dma_start(out=b2_t[:], in_=b2_d[:])
            # block-0 activations up front (gate the first matmuls); later
            # blocks' activations go at the END of the sync weight chain so
            # their transfers don't steal HBM bandwidth during the critical
            # W1-stream ramp (they aren't consumed until ~50us in).
            for k in range(KD):
                off, TB = blocks[0]
                nc.gpsimd.dma_start(out=xt_t[(0, k)][:],
                                    in_=xt_d[k * P:(k + 1) * P, off:off + TB])
            for bi, (off, TB) in enumerate(blocks[1:], start=1):
                for k in range(KD):
                    nc.sync.dma_start(out=xt_t[(bi, k)][:],
                                      in_=xt_d[k * P:(k + 1) * P, off:off + TB])
            def l1_group(bi, off, TB, ht_t, m):
                ps = ps1.tile([P, TBmax], F32, tag="ps1", name=f"ps1_{bi}_{m}")
                for k in range(KD):
                    nc.tensor.matmul(
                        ps[:, :TB],
                        w1_slice(m, k),
                        xt_t[(bi, k)][:],
                        start=(k == 0), stop=(k == KD - 1),
                    )
                nc.scalar.activation(
                    ht_t[m][:], ps[:, :TB],
                    mybir.ActivationFunctionType.Relu,
                    bias=b1_t[:, m:m + 1],
                )

            def l2_group(TB, ht_t, ps_o, m):
                for d in range(KD):
                    nc.tensor.matmul(
                        ps_o[d][:, :TB],
                        w2_t[m][:, d * P:(d + 1) * P],
                        ht_t[m][:],
                        start=(m == 0), stop=(m == MF - 1),
                    )

            for bi, (off, TB) in enumerate(blocks):
                ht_t = [cpool.tile([P, TB], WDT, tag=f"ht_{m}", name=f"ht_{bi}_{m}")
                        for m in range(MF)]
                if bi < last_bi:
                    # Streaming block: interleave layer-1 and layer-2 m-tiles
                    # (software-pipelined by one stage so the ReLU eviction
                    # hides) — spreads the W1+W2 DMA demand over the whole
                    # block so the PE isn't paced by the weight stream.
                    # Layer-2 accumulates into KD pinned PSUM groups.
                    ps_o = [ps2.tile([P, TBmax], F32, tag="ps2",
                                     name=f"ps2_{bi}_{d}") for d in range(KD)]
                    for m in range(MF):
                        l1_group(bi, off, TB, ht_t, m)
                        if m >= 1:
                            l2_group(TB, ht_t, ps_o, m - 1)
                    l2_group(TB, ht_t, ps_o, MF - 1)
                    for d in range(KD):
                        y_t = ypool.tile([P, TBmax], F32, tag="yt")
                        nc.vector.tensor_scalar_add(y_t[:, :TB], ps_o[d][:, :TB],
                                                    b2_t[:, d:d + 1])
                        nc.sync.dma_start(out=yt_d[d * P:(d + 1) * P, off:off + TB],
                                          in_=y_t[:, :TB])
                else:
                    # Final block (weights already resident): layer 1, then
                    # d-outer layer 2 so each d-group finishes early and the
                    # evictions + output DMAs overlap the remaining matmuls.
                    for m in range(MF):
                        l1_group(bi, off, TB, ht_t, m)
                    for d in range(KD):
                        ps_o = ps2.tile([P, TBmax], F32, tag="ps2",
                                        name=f"ps2L_{d}")
                        for m in range(MF):
                            nc.tensor.matmul(
                                ps_o[:, :TB],
                                w2_t[m][:, d * P:(d + 1) * P],
                                ht_t[m][:],
                                start=(m == 0), stop=(m == MF - 1),
                            )
                        y_t = ypool.tile([P, TBmax], F32, tag="yt")
                        nc.vector.tensor_scalar_add(y_t[:, :TB], ps_o[:, :TB],
                                                    b2_t[:, d:d + 1])
                        nc.sync.dma_start(out=yt_d[d * P:(d + 1) * P, off:off + TB],
                                          in_=y_t[:, :TB])

    nc.compile()
    _NC_CACHE[key] = nc
    return nc


def kernel(hidden_states, Wr, W1, b1, W2, b2):
    trace = os.environ.get("MOE_TRACE") == "1"
    wdt_name = os.environ.get("MOE_DTYPE", "bf16")
    if trace:
        _install_trace_shims()

    import ml_dtypes
    from concourse.bass_utils import run_bass_kernel_spmd

    wnp = np.float32 if wdt_name == "fp32r" else ml_dtypes.bfloat16

    X = np.ascontiguousarray(np.asarray(hidden_states, np.float32).reshape(B * S, D))
    Wr = np.asarray(Wr, np.float32)
    W1 = np.asarray(W1, np.float32)
    b1 = np.asarray(b1, np.float32)
    W2 = np.asarray(W2, np.float32)
    b2 = np.asarray(b2, np.float32)
    T = B * S

    # --- router (replicated; host) ---
    logits = X @ Wr                                   # [T, E] fp32
    expert_index = logits.argmax(-1)
    # top-1 softmax prob, computed the same way jax.nn.softmax does (max-shift)
    top = logits.max(-1, keepdims=True)
    prob = 1.0 / np.exp(logits - top, dtype=np.float32).sum(-1)  # [T]

    # --- dispatch: group tokens by expert ---
    order = np.argsort(expert_index, kind="stable")
    sorted_eidx = expert_index[order]
    bounds = np.searchsorted(sorted_eidx, np.arange(E + 1))
    counts = np.diff(bounds)
    C = max(256, int(counts.max()))
    blocks = _blocks_for(C)

    Xs = X[order]
    in_maps = []
    for i in range(E):
        lo, hi = bounds[i], bounds[i + 1]
        xt = np.zeros((D, C), wnp)
        xt[:, : hi - lo] = Xs[lo:hi].T
        # m-major swizzle of W1: [D, F] -> [MF, 128, KD*128]
        w1m = np.ascontiguousarray(
            W1[i].reshape(KD, P, MF, P).transpose(2, 1, 0, 3).reshape(MF, P, KD * P)
        ).astype(wnp)
        in_maps.append({
            "xt": xt,
            "w1m": w1m,
            "b1": np.ascontiguousarray(b1[i].reshape(MF, P).T),
            "w2": np.ascontiguousarray(W2[i]).astype(wnp),
            "b2": np.ascontiguousarray(b2[i].reshape(KD, P).T),
        })

    # --- expert FFNs on the 8 cores ---
    nc = _build_device_kernel(C, blocks, wdt_name)
    res = run_bass_kernel_spmd(nc, in_maps, core_ids=list(range(E)), trace=trace)
    if trace:
        LAST_PROFILE.clear()
        LAST_PROFILE.update(
            exec_time_ns=res.exec_time_ns,
            mean_exec_time_ns=res.mean_exec_time_ns,
            max_exec_time_core_id=res.max_exec_time_core_id,
            trace_path=(res.instructions_and_trace or (None, None))[1],
            profile_json=res.profile_json,
            capacity=C,
            blocks=blocks,
            counts=counts.tolist(),
            dtype=wdt_name,
        )

    # --- combine: un-sort, scale by router prob ---
    Ys = np.empty((T, D), np.float32)
    for i in range(E):
        lo, hi = bounds[i], bounds[i + 1]
        Ys[lo:hi] = res.results[i]["yt"][:, : hi - lo].T
    combined = np.empty_like(Ys)
    combined[order] = Ys
    out = (prob[:, None] * combined).reshape(B, S, D)

    return (
        out,
        logits.reshape(B, S, E),
        expert_index.reshape(B, S).astype(np.int32),
    )

